# revision 4
# baseline (speedup 1.0000x reference)
"""Distributed Trainium2 kernel for a single attention head.

Problem: x:[8,2048,1024] f32, w_q/w_k/w_v:[1024,64] f32
  q,k,v = x@w ; scores = (q k^T)/sqrt(1024) causal-masked; out = softmax(scores)@v

Sharding: data-parallel over batch B=8 across the 8 NeuronCores (one batch
element per core, weights replicated, no collectives).

Per-core dataflow (T=2048, C=1024, H=64):
  - host ships x^T [C,T] in bf16 (layout marshalling), packed w_qk [C,128],
    w_v (bf16), a triangular mask tile, and identities for TensorE transposes.
  - projections with weights stationary (bf16): qT,kT duplicated on both
    partition halves [128,T] so scores can run 2x row-tiled; vT [64,T].
  - scores computed TRANSPOSED per s-tile: S[s,t] = kT_slice.T @ qT (K=64)
    so the PV contraction over s has s on partitions. Two s-tiles run
    concurrently in PE row-groups 0/1 (tile_position row packing).
  - exp on ScalarE with scale=1/32 folded in (no max-subtraction needed:
    |scores|<~2), output cast to bf16.
  - causal: only j<=t blocks computed; triangular mask multiply on diagonal
    blocks; memset-zero for fully-masked ranges.
  - PV: out^T[h,t] accumulated over s-tiles with lhsT = [v | 1] so row 64 of
    the accumulator is the softmax denominator (fused row-sum).
  - epilogue: TensorE transpose back to [t,h], multiply by reciprocal
    denominator on VectorE, DMA out (f32).
"""

import os
import sys

import numpy as np

for p in ("/opt/trn_rl_repo",):
    if p not in sys.path and os.path.isdir(p):
        sys.path.insert(0, p)

import ml_dtypes  # noqa: E402

B, T, C, H = 8, 2048, 1024, 64
N_CORES = 8
TCH = 512                  # t-chunk (columns per PSUM bank of f32)
N_CHUNK = T // TCH         # 4
N_ST = T // 128            # 16 s-tiles
SCALE = float(C) ** -0.5   # 1/32

_CACHE = {}


def _build():
    """Build + compile the SPMD Bass graph (same graph on all 8 cores)."""
    import concourse.bass as bass
    import concourse.mybir as mybir
    import concourse.tile as tile
    from concourse import bacc

    f32 = mybir.dt.float32
    bf16 = mybir.dt.bfloat16
    EXP = mybir.ActivationFunctionType.Exp

    nc = bacc.Bacc(
        "TRN2", target_bir_lowering=False, debug=False, num_devices=N_CORES
    )

    xT_d = nc.dram_tensor("xT", [C, T], bf16, kind="ExternalInput")
    wqk_d = nc.dram_tensor("wqk", [C, 128], bf16, kind="ExternalInput")
    wv_d = nc.dram_tensor("wv", [C, H], bf16, kind="ExternalInput")
    mask_d = nc.dram_tensor("maskb", [128, 128], bf16, kind="ExternalInput")
    idf_d = nc.dram_tensor("idf", [128, 128], f32, kind="ExternalInput")
    idb_d = nc.dram_tensor("idb", [128, 128], bf16, kind="ExternalInput")
    out_d = nc.dram_tensor("out", [T, H], f32, kind="ExternalOutput")

    with tile.TileContext(nc) as tc:
        with (
            tc.tile_pool(name="const", bufs=1) as constp,
            tc.tile_pool(name="xTp", bufs=1) as xTp,
            tc.tile_pool(name="qkp", bufs=1) as qkp,
            tc.tile_pool(name="v1p", bufs=1) as v1p,
            tc.tile_pool(name="exp", bufs=4) as expp,
            tc.tile_pool(name="epi", bufs=3) as epip,
            tc.tile_pool(name="Sp", bufs=2, space="PSUM") as Sp,
            tc.tile_pool(name="accp", bufs=2, space="PSUM") as accp,
            tc.tile_pool(name="miscp", bufs=2, space="PSUM") as miscp,
        ):
            # ---- constants ----
            wqk_t = constp.tile([128, C // 128, 128], bf16, tag="wqk", name="wqk_t")
            nc.sync.dma_start(
                out=wqk_t[:], in_=wqk_d[:].rearrange("(n p) m -> p n m", p=128)
            )
            wv_t = constp.tile([128, C // 128, H], bf16, tag="wv", name="wv_t")
            nc.sync.dma_start(
                out=wv_t[:], in_=wv_d[:].rearrange("(n p) m -> p n m", p=128)
            )
            mask_t = constp.tile([128, 128], bf16, tag="mask", name="mask_t")
            nc.sync.dma_start(out=mask_t[:], in_=mask_d[:])
            idf_t = constp.tile([128, 128], f32, tag="idf", name="idf_t")
            nc.sync.dma_start(out=idf_t[:], in_=idf_d[:])
            idb_t = constp.tile([128, 128], bf16, tag="idb", name="idb_t")
            nc.sync.dma_start(out=idb_t[:], in_=idb_d[:])

            # ---- x^T input tiles, chunk-major so chunk 0 lands first ----
            xt = {}
            for t in range(N_CHUNK):
                for c in range(C // 128):
                    xx = xTp.tile(
                        [128, TCH], bf16, tag=f"x{c}_{t}", name=f"x{c}_{t}"
                    )
                    nc.sync.dma_start(
                        out=xx[:],
                        in_=xT_d[128 * c : 128 * (c + 1), TCH * t : TCH * (t + 1)],
                    )
                    xt[c, t] = xx

            qk2 = {}   # [128, TCH]: qT duplicated on both partition halves
            kk2 = {}   # [128, TCH]: kT duplicated on both partition halves
            v1 = {}

            for tch in range(N_CHUNK):
                # ======== phase A: projections for this t-chunk ========
                S = Sp.tile([128, 2 * TCH], f32, tag="S", name=f"Sqk{tch}")
                for c in range(C // 128):
                    nc.tensor.matmul(
                        S[:, 0:TCH],
                        wqk_t[:, c, :],
                        xt[c, tch][:],
                        start=(c == 0),
                        stop=(c == C // 128 - 1),
                    )
                q2 = qkp.tile([128, TCH], bf16, tag=f"q2_{tch}", name=f"q2_{tch}")
                k2 = qkp.tile([128, TCH], bf16, tag=f"k2_{tch}", name=f"k2_{tch}")
                nc.vector.tensor_copy(q2[0:64, :], S[0:64, 0:TCH])
                nc.vector.tensor_copy(k2[0:64, :], S[64:128, 0:TCH])
                # duplicate to upper partition half for row-group-1 matmuls
                nc.sync.dma_start(out=q2[64:128, :], in_=q2[0:64, :])
                nc.sync.dma_start(out=k2[64:128, :], in_=k2[0:64, :])
                qk2[tch] = q2
                kk2[tch] = k2

                Pv = miscp.tile([128, TCH], f32, tag="misc", name=f"Pv{tch}")
                for c in range(C // 128):
                    nc.tensor.matmul(
                        Pv[0:64, :],
                        wv_t[:, c, :],
                        xt[c, tch][:],
                        start=(c == 0),
                        stop=(c == C // 128 - 1),
                    )
                vTt = qkp.tile([64, TCH], bf16, tag=f"vT{tch}", name=f"vT{tch}")
                nc.vector.tensor_copy(vTt[:], Pv[0:64, :])

                # build v1 tiles [128, 65] = [v | ones] for the 4 s-tiles
                for i in range(4):
                    j = 4 * tch + i
                    Pt = miscp.tile(
                        [128, TCH], bf16, tag="misc", name=f"Pt{j}"
                    )
                    nc.tensor.transpose(
                        Pt[0:128, 0:64],
                        vTt[:, 128 * i : 128 * (i + 1)],
                        idb_t[0:64, 0:64],
                    )
                    v1t = v1p.tile([128, 65], bf16, tag=f"v1_{j}", name=f"v1_{j}")
                    nc.vector.tensor_copy(v1t[:, 0:64], Pt[0:128, 0:64])
                    nc.vector.memset(v1t[:, 64:65], 1.0)
                    v1[j] = v1t

                # ======== phase B: scores/softmax/PV for this t-chunk ========
                jmax = 4 * tch + 3
                acc = accp.tile([65, TCH], f32, tag="acc", name=f"acc{tch}")
                for jp in range(0, jmax + 1, 2):
                    S2 = Sp.tile([128, 2 * TCH], f32, tag="S", name=f"S{tch}_{jp}")
                    for jj in range(2):
                        j = jp + jj
                        half = slice(64 * jj, 64 * (jj + 1))
                        ksl = kk2[j // 4][half, 128 * (j % 4) : 128 * (j % 4 + 1)]
                        lo = 128 * max(0, j - 4 * tch)  # causal: cols < lo masked
                        nc.tensor.matmul(
                            S2[:, TCH * jj + lo : TCH * (jj + 1)],
                            ksl,
                            qk2[tch][half, lo:TCH],
                            start=True,
                            stop=True,
                        )
                    ext = expp.tile([128, 2 * TCH], bf16, tag="ex", name=f"ex{tch}_{jp}")
                    if jp >= 4 * tch:
                        # diagonal pair: exp each half over its valid range only
                        for jj in range(2):
                            j = jp + jj
                            lo = TCH * jj + 128 * max(0, j - 4 * tch)
                            hi = TCH * (jj + 1)
                            nc.scalar.activation(
                                ext[:, lo:hi], S2[:, lo:hi], EXP, scale=SCALE
                            )
                    else:
                        nc.scalar.activation(ext[:], S2[:], EXP, scale=SCALE)
                    # causal: triangular mask multiply on diagonal blocks
                    # (fully-masked ranges are simply never read by PV below)
                    for jj in range(2):
                        j = jp + jj
                        rel = j - 4 * tch
                        if rel >= 0:
                            a = TCH * jj + 128 * rel
                            nc.vector.tensor_mul(
                                ext[:, a : a + 128], ext[:, a : a + 128], mask_t[:]
                            )
                    # PV accumulation (adds softmax-denominator row via ones col)
                    for jj in range(2):
                        j = jp + jj
                        lo = 128 * max(0, j - 4 * tch)
                        nc.tensor.matmul(
                            acc[:, lo:TCH] if j > 0 else acc[:, :],
                            v1[j][:],
                            ext[:, TCH * jj + lo : TCH * (jj + 1)],
                            start=(j == 0),
                            stop=(j == jmax),
                            skip_group_check=True,
                        )

                # ======== epilogue: normalize + transpose + DMA out ========
                oT = epip.tile([65, TCH], f32, tag="oT", name=f"oT{tch}")
                nc.vector.tensor_copy(oT[:], acc[:])
                for i in range(4):
                    Pe = miscp.tile([128, TCH], f32, tag="misc", name=f"Pe{tch}_{i}")
                    nc.tensor.transpose(
                        Pe[0:128, 0:65],
                        oT[:, 128 * i : 128 * (i + 1)],
                        idf_t[0:65, 0:65],
                    )
                    rec = epip.tile([128, 1], f32, tag="rec", name=f"rec{tch}_{i}")
                    nc.vector.reciprocal(rec[:], Pe[0:128, 64:65])
                    ot = epip.tile([128, H], f32, tag="ot", name=f"ot{tch}_{i}")
                    nc.vector.tensor_scalar_mul(ot[:], Pe[0:128, 0:64], rec[:])
                    r0 = TCH * tch + 128 * i
                    nc.sync.dma_start(out=out_d[r0 : r0 + 128, :], in_=ot[:])

    nc.compile()
    return nc


def _get_nc():
    if "nc" not in _CACHE:
        _CACHE["nc"] = _build()
    return _CACHE["nc"]


def _host_inputs(x, w_q, w_k, w_v):
    bf = ml_dtypes.bfloat16
    x = np.asarray(x, dtype=np.float32)
    wqk = np.ascontiguousarray(
        np.concatenate([np.asarray(w_q, np.float32), np.asarray(w_k, np.float32)], 1)
    ).astype(bf)
    wv = np.ascontiguousarray(np.asarray(w_v, np.float32)).astype(bf)
    mask = np.triu(np.ones((128, 128), np.float32)).astype(bf)
    idf = np.eye(128, dtype=np.float32)
    idb = np.eye(128, dtype=np.float32).astype(bf)
    in_maps = []
    for i in range(N_CORES):
        in_maps.append(
            {
                "xT": np.ascontiguousarray(x[i].T).astype(bf),
                "wqk": wqk,
                "wv": wv,
                "maskb": mask,
                "idf": idf,
                "idb": idb,
            }
        )
    return in_maps


def run(x, w_q, w_k, w_v, trace=False, **trace_kwargs):
    from concourse.bass_utils import run_bass_kernel_spmd

    nc = _get_nc()
    in_maps = _host_inputs(x, w_q, w_k, w_v)
    res = run_bass_kernel_spmd(
        nc, in_maps, core_ids=list(range(N_CORES)), trace=trace, **trace_kwargs
    )
    out = np.stack([np.asarray(res.results[i]["out"]) for i in range(N_CORES)])
    return out.astype(np.float32), res


def kernel(x, w_q, w_k, w_v):
    out, _ = run(x, w_q, w_k, w_v, trace=False)
    return out


# revision 9
# speedup vs baseline: 1.1847x; 1.1847x over previous
"""Distributed Trainium2 kernel for a single attention head.

Problem: x:[8,2048,1024] f32, w_q/w_k/w_v:[1024,64] f32
  q,k,v = x@w ; scores = (q k^T)/sqrt(1024) causal-masked; out = softmax(scores)@v

Sharding: data-parallel over batch B=8 across the 8 NeuronCores (one batch
element per core, weights replicated, no collectives).

Per-core dataflow (T=2048, C=1024, H=64):
  - host ships x^T [C,T] in bf16 (layout marshalling), packed w_qk [C,128],
    w_v (bf16), a triangular mask tile, and identities for TensorE transposes.
  - projections with weights stationary (bf16): qT,kT duplicated on both
    partition halves [128,T] so scores can run 2x row-tiled; vT [64,T].
  - scores computed TRANSPOSED per s-tile: S[s,t] = kT_slice.T @ qT (K=64)
    so the PV contraction over s has s on partitions. Two s-tiles run
    concurrently in PE row-groups 0/1 (tile_position row packing).
  - exp on ScalarE with scale=1/32 folded in (no max-subtraction needed:
    |scores|<~2), output cast to bf16.
  - causal: only j<=t blocks computed; triangular mask multiply on diagonal
    blocks; memset-zero for fully-masked ranges.
  - PV: out^T[h,t] accumulated over s-tiles with lhsT = [v | 1] so row 64 of
    the accumulator is the softmax denominator (fused row-sum).
  - epilogue: TensorE transpose back to [t,h], multiply by reciprocal
    denominator on VectorE, DMA out (f32).
"""

import os
import sys

import numpy as np

for p in ("/opt/trn_rl_repo",):
    if p not in sys.path and os.path.isdir(p):
        sys.path.insert(0, p)

import ml_dtypes  # noqa: E402

B, T, C, H = 8, 2048, 1024, 64
N_CORES = 8
TCH = 512                  # t-chunk (columns per PSUM bank of f32)
N_CHUNK = T // TCH         # 4
N_ST = T // 128            # 16 s-tiles
SCALE = float(C) ** -0.5   # 1/32

_CACHE = {}


def _build():
    """Build + compile the SPMD Bass graph (same graph on all 8 cores)."""
    import concourse.bass as bass
    import concourse.mybir as mybir
    import concourse.tile as tile
    from concourse import bacc

    f32 = mybir.dt.float32
    bf16 = mybir.dt.bfloat16
    EXP = mybir.ActivationFunctionType.Exp

    nc = bacc.Bacc(
        "TRN2", target_bir_lowering=False, debug=False, num_devices=N_CORES
    )

    xT_d = nc.dram_tensor("xT", [C, T], bf16, kind="ExternalInput")
    wqk_d = nc.dram_tensor("wqk", [C, 128], bf16, kind="ExternalInput")
    wv_d = nc.dram_tensor("wv", [C, H], bf16, kind="ExternalInput")
    mask_d = nc.dram_tensor("maskb", [128, 128], bf16, kind="ExternalInput")
    idf_d = nc.dram_tensor("idf", [128, 128], f32, kind="ExternalInput")
    idb_d = nc.dram_tensor("idb", [128, 128], bf16, kind="ExternalInput")
    out_d = nc.dram_tensor("out", [T, H], f32, kind="ExternalOutput")

    with tile.TileContext(nc) as tc:
        with (
            tc.tile_pool(name="const", bufs=1) as constp,
            tc.tile_pool(name="xTp", bufs=1) as xTp,
            tc.tile_pool(name="qkp", bufs=1) as qkp,
            tc.tile_pool(name="v1p", bufs=1) as v1p,
            tc.tile_pool(name="exp", bufs=6) as expp,
            tc.tile_pool(name="epi", bufs=3) as epip,
            tc.tile_pool(name="Sp", bufs=2, space="PSUM") as Sp,
            tc.tile_pool(name="accp", bufs=1, space="PSUM") as accp,
            tc.tile_pool(name="miscp", bufs=3, space="PSUM") as miscp,
        ):
            # ---- constants (spread across DMA queues) ----
            wqk_t = constp.tile([128, C // 128, 128], bf16, tag="wqk", name="wqk_t")
            nc.sync.dma_start(
                out=wqk_t[:], in_=wqk_d[:].rearrange("(n p) m -> p n m", p=128)
            )
            wv_t = constp.tile([128, C // 128, H], bf16, tag="wv", name="wv_t")
            nc.scalar.dma_start(
                out=wv_t[:], in_=wv_d[:].rearrange("(n p) m -> p n m", p=128)
            )
            mask_t = constp.tile([128, 128], bf16, tag="mask", name="mask_t")
            nc.gpsimd.dma_start(out=mask_t[:], in_=mask_d[:])
            idf_t = constp.tile([128, 128], f32, tag="idf", name="idf_t")
            nc.gpsimd.dma_start(out=idf_t[:], in_=idf_d[:])
            idb_t = constp.tile([128, 128], bf16, tag="idb", name="idb_t")
            nc.gpsimd.dma_start(out=idb_t[:], in_=idb_d[:])

            # ---- x^T input tiles, chunk-major so chunk 0 lands first;
            # round-robin the issuing engine to use 3 parallel DMA queues ----
            dma_engines = [nc.sync, nc.gpsimd, nc.scalar]
            xt = {}
            for t in range(N_CHUNK):
                for c in range(C // 128):
                    xx = xTp.tile(
                        [128, TCH], bf16, tag=f"x{c}_{t}", name=f"x{c}_{t}"
                    )
                    eng = dma_engines[(t * (C // 128) + c) % 3]
                    eng.dma_start(
                        out=xx[:],
                        in_=xT_d[128 * c : 128 * (c + 1), TCH * t : TCH * (t + 1)],
                    )
                    xt[c, t] = xx

            qk2 = {}   # [128, TCH]: qT duplicated on both partition halves
            kk2 = {}   # [128, TCH]: kT duplicated on both partition halves
            v1 = {}

            for tch in range(N_CHUNK):
                # ======== phase A: projections for this t-chunk ========
                S = miscp.tile([128, TCH], f32, tag="misc", name=f"Sqk{tch}")
                for c in range(C // 128):
                    nc.tensor.matmul(
                        S[:, :],
                        wqk_t[:, c, :],
                        xt[c, tch][:],
                        start=(c == 0),
                        stop=(c == C // 128 - 1),
                    )
                q2 = qkp.tile([128, TCH], bf16, tag=f"q2_{tch}", name=f"q2_{tch}")
                k2 = qkp.tile([128, TCH], bf16, tag=f"k2_{tch}", name=f"k2_{tch}")
                nc.vector.tensor_copy(q2[0:64, :], S[0:64, :])
                nc.vector.tensor_copy(k2[0:64, :], S[64:128, :])
                # duplicate to upper partition half for row-group-1 matmuls
                nc.gpsimd.dma_start(out=q2[64:128, :], in_=q2[0:64, :])
                nc.gpsimd.dma_start(out=k2[64:128, :], in_=k2[0:64, :])
                qk2[tch] = q2
                kk2[tch] = k2

                Pv = miscp.tile([128, TCH], f32, tag="misc", name=f"Pv{tch}")
                for c in range(C // 128):
                    nc.tensor.matmul(
                        Pv[0:64, :],
                        wv_t[:, c, :],
                        xt[c, tch][:],
                        start=(c == 0),
                        stop=(c == C // 128 - 1),
                    )
                vTt = qkp.tile([64, TCH], bf16, tag=f"vT{tch}", name=f"vT{tch}")
                nc.vector.tensor_copy(vTt[:], Pv[0:64, :])

                # build v1 tiles [128, 65] = [v | ones] for the 4 s-tiles
                for i in range(4):
                    j = 4 * tch + i
                    Pt = miscp.tile(
                        [128, TCH], bf16, tag="misc", name=f"Pt{j}"
                    )
                    nc.tensor.transpose(
                        Pt[0:128, 0:64],
                        vTt[:, 128 * i : 128 * (i + 1)],
                        idb_t[0:64, 0:64],
                    )
                    v1t = v1p.tile([128, 65], bf16, tag=f"v1_{j}", name=f"v1_{j}")
                    nc.vector.tensor_copy(v1t[:, 0:64], Pt[0:128, 0:64])
                    nc.vector.memset(v1t[:, 64:65], 1.0)
                    v1[j] = v1t

                # ======== phase B: scores/softmax/PV for this t-chunk ========
                jmax = 4 * tch + 3
                acc = accp.tile([65, TCH], f32, tag="acc", name=f"acc{tch}")
                for jp in range(0, jmax + 1, 2):
                    S2 = Sp.tile([128, 2 * TCH], f32, tag="S", name=f"S{tch}_{jp}")
                    for jj in range(2):
                        j = jp + jj
                        half = slice(64 * jj, 64 * (jj + 1))
                        ksl = kk2[j // 4][half, 128 * (j % 4) : 128 * (j % 4 + 1)]
                        lo = 128 * max(0, j - 4 * tch)  # causal: cols < lo masked
                        nc.tensor.matmul(
                            S2[:, TCH * jj + lo : TCH * (jj + 1)],
                            ksl,
                            qk2[tch][half, lo:TCH],
                            start=True,
                            stop=True,
                        )
                    ext = expp.tile([128, 2 * TCH], bf16, tag="ex", name=f"ex{tch}_{jp}")
                    if jp >= 4 * tch:
                        # diagonal pair: exp each half over its valid range only
                        for jj in range(2):
                            j = jp + jj
                            lo = TCH * jj + 128 * max(0, j - 4 * tch)
                            hi = TCH * (jj + 1)
                            nc.scalar.activation(
                                ext[:, lo:hi], S2[:, lo:hi], EXP, scale=SCALE
                            )
                    else:
                        nc.scalar.activation(ext[:], S2[:], EXP, scale=SCALE)
                    # causal: triangular mask multiply on diagonal blocks
                    # (fully-masked ranges are simply never read by PV below)
                    for jj in range(2):
                        j = jp + jj
                        rel = j - 4 * tch
                        if rel >= 0:
                            a = TCH * jj + 128 * rel
                            nc.gpsimd.tensor_mul(
                                ext[:, a : a + 128], ext[:, a : a + 128], mask_t[:]
                            )
                    # PV accumulation (adds softmax-denominator row via ones col)
                    for jj in range(2):
                        j = jp + jj
                        lo = 128 * max(0, j - 4 * tch)
                        nc.tensor.matmul(
                            acc[:, lo:TCH] if j > 0 else acc[:, :],
                            v1[j][:],
                            ext[:, TCH * jj + lo : TCH * (jj + 1)],
                            start=(j == 0),
                            stop=(j == jmax),
                            skip_group_check=True,
                        )

                # ======== epilogue: normalize + transpose + DMA out ========
                oT = epip.tile([65, TCH], f32, tag="oT", name=f"oT{tch}")
                nc.vector.tensor_copy(oT[:], acc[:])
                for i in range(4):
                    Pe = miscp.tile([128, TCH], f32, tag="misc", name=f"Pe{tch}_{i}")
                    nc.tensor.transpose(
                        Pe[0:128, 0:65],
                        oT[:, 128 * i : 128 * (i + 1)],
                        idf_t[0:65, 0:65],
                    )
                    rec = epip.tile([128, 1], f32, tag="rec", name=f"rec{tch}_{i}")
                    nc.vector.reciprocal(rec[:], Pe[0:128, 64:65])
                    ot = epip.tile([128, H], f32, tag="ot", name=f"ot{tch}_{i}")
                    nc.vector.tensor_scalar_mul(ot[:], Pe[0:128, 0:64], rec[:])
                    r0 = TCH * tch + 128 * i
                    nc.sync.dma_start(out=out_d[r0 : r0 + 128, :], in_=ot[:])

    nc.compile()
    return nc


def _get_nc():
    if "nc" not in _CACHE:
        _CACHE["nc"] = _build()
    return _CACHE["nc"]


def _host_inputs(x, w_q, w_k, w_v):
    bf = ml_dtypes.bfloat16
    x = np.asarray(x, dtype=np.float32)
    wqk = np.ascontiguousarray(
        np.concatenate([np.asarray(w_q, np.float32), np.asarray(w_k, np.float32)], 1)
    ).astype(bf)
    wv = np.ascontiguousarray(np.asarray(w_v, np.float32)).astype(bf)
    mask = np.triu(np.ones((128, 128), np.float32)).astype(bf)
    idf = np.eye(128, dtype=np.float32)
    idb = np.eye(128, dtype=np.float32).astype(bf)
    in_maps = []
    for i in range(N_CORES):
        in_maps.append(
            {
                "xT": np.ascontiguousarray(x[i].T).astype(bf),
                "wqk": wqk,
                "wv": wv,
                "maskb": mask,
                "idf": idf,
                "idb": idb,
            }
        )
    return in_maps


def run(x, w_q, w_k, w_v, trace=False, **trace_kwargs):
    from concourse.bass_utils import run_bass_kernel_spmd

    nc = _get_nc()
    in_maps = _host_inputs(x, w_q, w_k, w_v)
    res = run_bass_kernel_spmd(
        nc, in_maps, core_ids=list(range(N_CORES)), trace=trace, **trace_kwargs
    )
    out = np.stack([np.asarray(res.results[i]["out"]) for i in range(N_CORES)])
    return out.astype(np.float32), res


def kernel(x, w_q, w_k, w_v):
    out, _ = run(x, w_q, w_k, w_v, trace=False)
    return out


# revision 10
# speedup vs baseline: 1.3854x; 1.1694x over previous
"""Distributed Trainium2 kernel for a single attention head.

Problem: x:[8,2048,1024] f32, w_q/w_k/w_v:[1024,64] f32
  q,k,v = x@w ; scores = (q k^T)/sqrt(1024) causal-masked; out = softmax(scores)@v

Sharding: data-parallel over batch B=8 across the 8 NeuronCores (one batch
element per core, weights replicated, no collectives).

Per-core dataflow (T=2048, C=1024, H=64):
  - host ships x^T [C,T] in bf16 (layout marshalling), pre-tiled packed
    w_qk / w_v (bf16), a triangular mask tile, and identities for transposes.
  - projections with weights stationary (bf16): qT,kT duplicated on both
    partition halves [128,T] so scores can run 2x row-tiled; vT [64,T].
  - scores computed TRANSPOSED per s-tile: S[s,t] = kT_slice.T @ qT (K=64)
    so the PV contraction over s has s on partitions. Two s-tiles run
    concurrently in PE row-groups 0/1 (tile_position row packing).
  - exp on ScalarE with scale=1/32 folded in (no max-subtraction needed:
    |scores|<~2), output cast to bf16.
  - causal: only j<=t blocks computed; triangular mask multiply on diagonal
    blocks runs on GpSimd (otherwise idle).
  - PV: out^T[h,t] accumulated over s-tiles with lhsT = [v | 1] so row 64 of
    the accumulator is the softmax denominator (fused row-sum).
  - epilogue: TensorE transpose back to [t,h], multiply by reciprocal
    denominator on VectorE, DMA out (f32).
  - next-chunk projections are emission-interleaved between attention pairs
    so the PE stream stays dense (HAM stays warm) and input DMAs are spread
    over the 3 DMA-capable queues (sync/gpsimd/scalar).
"""

import os
import sys

import numpy as np

for p in ("/opt/trn_rl_repo",):
    if p not in sys.path and os.path.isdir(p):
        sys.path.insert(0, p)

import ml_dtypes  # noqa: E402

B, T, C, H = 8, 2048, 1024, 64
N_CORES = 8
TCH = 512                  # t-chunk (columns per PSUM bank of f32)
N_CHUNK = T // TCH         # 4
N_CT = C // 128            # 8 contraction tiles
SCALE = float(C) ** -0.5   # 1/32

_CACHE = {}


def _build():
    """Build + compile the SPMD Bass graph (same graph on all 8 cores)."""
    import concourse.bass as bass
    import concourse.mybir as mybir
    import concourse.tile as tile
    from concourse import bacc

    f32 = mybir.dt.float32
    bf16 = mybir.dt.bfloat16
    EXP = mybir.ActivationFunctionType.Exp

    nc = bacc.Bacc(
        "TRN2", target_bir_lowering=False, debug=False, num_devices=N_CORES
    )

    # host ships weights pre-tiled: [128, N_CT*free] with c-tile-major columns
    xT_d = nc.dram_tensor("xT", [C, T], bf16, kind="ExternalInput")
    wqk_d = nc.dram_tensor("wqk", [128, N_CT * 128], bf16, kind="ExternalInput")
    wv_d = nc.dram_tensor("wv", [128, N_CT * H], bf16, kind="ExternalInput")
    mask_d = nc.dram_tensor("maskb", [128, 128], bf16, kind="ExternalInput")
    idf_d = nc.dram_tensor("idf", [128, 128], f32, kind="ExternalInput")
    idb_d = nc.dram_tensor("idb", [128, 128], bf16, kind="ExternalInput")
    out_d = nc.dram_tensor("out", [T, H], f32, kind="ExternalOutput")

    with tile.TileContext(nc) as tc:
        with (
            tc.tile_pool(name="const", bufs=1) as constp,
            tc.tile_pool(name="xTp", bufs=1) as xTp,
            tc.tile_pool(name="qkp", bufs=1) as qkp,
            tc.tile_pool(name="v1p", bufs=1) as v1p,
            tc.tile_pool(name="exp", bufs=6) as expp,
            tc.tile_pool(name="epi", bufs=3) as epip,
            tc.tile_pool(name="Sp", bufs=2, space="PSUM") as Sp,
            tc.tile_pool(name="accp", bufs=1, space="PSUM") as accp,
            tc.tile_pool(name="miscp", bufs=3, space="PSUM") as miscp,
        ):
            # ---- weights first (contiguous per-partition layout) ----
            wqk_t = constp.tile([128, N_CT, 128], bf16, tag="wqk", name="wqk_t")
            nc.sync.dma_start(
                out=wqk_t[:], in_=wqk_d[:].rearrange("p (n m) -> p n m", n=N_CT)
            )
            wv_t = constp.tile([128, N_CT, H], bf16, tag="wv", name="wv_t")
            nc.scalar.dma_start(
                out=wv_t[:], in_=wv_d[:].rearrange("p (n m) -> p n m", n=N_CT)
            )

            # ---- x^T tiles: chunk 0 first, 3 parallel DMA queues ----
            dma_engines = [nc.sync, nc.gpsimd, nc.scalar]
            xt = {}

            def emit_x_dma(t, c):
                xx = xTp.tile([128, TCH], bf16, tag=f"x{c}_{t}", name=f"x{c}_{t}")
                eng = dma_engines[(t * N_CT + c) % 3]
                eng.dma_start(
                    out=xx[:],
                    in_=xT_d[128 * c : 128 * (c + 1), TCH * t : TCH * (t + 1)],
                )
                xt[c, t] = xx

            for c in range(N_CT):
                emit_x_dma(0, c)

            # small constants after chunk-0 x
            mask_t = constp.tile([128, 128], bf16, tag="mask", name="mask_t")
            nc.gpsimd.dma_start(out=mask_t[:], in_=mask_d[:])
            idf_t = constp.tile([128, 128], f32, tag="idf", name="idf_t")
            nc.gpsimd.dma_start(out=idf_t[:], in_=idf_d[:])
            idb_t = constp.tile([128, 128], bf16, tag="idb", name="idb_t")
            nc.gpsimd.dma_start(out=idb_t[:], in_=idb_d[:])

            for t in range(1, N_CHUNK):
                for c in range(N_CT):
                    emit_x_dma(t, c)

            qk2 = {}   # [128, TCH]: qT duplicated on both partition halves
            kk2 = {}   # [128, TCH]: kT duplicated on both partition halves
            v1 = {}

            def proj_steps(tch):
                """Emission thunks for chunk `tch` projections + v1 build."""
                steps = []
                state = {}

                def qk_mm(c):
                    def f():
                        if c == 0:
                            state["S"] = miscp.tile(
                                [128, TCH], f32, tag="misc", name=f"Sqk{tch}"
                            )
                        nc.tensor.matmul(
                            state["S"][:, :],
                            wqk_t[:, c, :],
                            xt[c, tch][:],
                            start=(c == 0),
                            stop=(c == N_CT - 1),
                            skip_group_check=True,
                        )
                    return f

                def qk_out():
                    S = state["S"]
                    q2 = qkp.tile([128, TCH], bf16, tag=f"q2_{tch}", name=f"q2_{tch}")
                    k2 = qkp.tile([128, TCH], bf16, tag=f"k2_{tch}", name=f"k2_{tch}")
                    nc.vector.tensor_copy(q2[0:64, :], S[0:64, :])
                    nc.vector.tensor_copy(k2[0:64, :], S[64:128, :])
                    nc.gpsimd.dma_start(out=q2[64:128, :], in_=q2[0:64, :])
                    nc.gpsimd.dma_start(out=k2[64:128, :], in_=k2[0:64, :])
                    qk2[tch] = q2
                    kk2[tch] = k2

                def v_mm(c):
                    def f():
                        if c == 0:
                            state["Pv"] = miscp.tile(
                                [128, TCH], f32, tag="misc", name=f"Pv{tch}"
                            )
                        nc.tensor.matmul(
                            state["Pv"][0:64, :],
                            wv_t[:, c, :],
                            xt[c, tch][:],
                            start=(c == 0),
                            stop=(c == N_CT - 1),
                            skip_group_check=True,
                        )
                    return f

                def v_out():
                    vTt = qkp.tile([64, TCH], bf16, tag=f"vT{tch}", name=f"vT{tch}")
                    nc.vector.tensor_copy(vTt[:], state["Pv"][0:64, :])
                    state["vT"] = vTt

                def v1_build(i):
                    def f():
                        j = 4 * tch + i
                        Pt = miscp.tile([128, TCH], bf16, tag="misc", name=f"Pt{j}")
                        nc.tensor.transpose(
                            Pt[0:128, 0:64],
                            state["vT"][:, 128 * i : 128 * (i + 1)],
                            idb_t[0:64, 0:64],
                        )
                        v1t = v1p.tile(
                            [128, 65], bf16, tag=f"v1_{j}", name=f"v1_{j}"
                        )
                        nc.vector.tensor_copy(v1t[:, 0:64], Pt[0:128, 0:64])
                        nc.vector.memset(v1t[:, 64:65], 1.0)
                        v1[j] = v1t
                    return f

                for c in range(N_CT):
                    steps.append(qk_mm(c))
                steps.append(qk_out)
                for c in range(N_CT):
                    steps.append(v_mm(c))
                steps.append(v_out)
                for i in range(4):
                    steps.append(v1_build(i))
                return steps

            # chunk 0 projections up front
            for s in proj_steps(0):
                s()

            for tch in range(N_CHUNK):
                # interleave next chunk's projection emission between pairs
                pending = proj_steps(tch + 1) if tch + 1 < N_CHUNK else []
                jmax = 4 * tch + 3
                pairs = list(range(0, jmax + 1, 2))
                per_pair = -(-len(pending) // len(pairs)) if pending else 0

                acc = accp.tile([65, TCH], f32, tag="acc", name=f"acc{tch}")
                for pi, jp in enumerate(pairs):
                    S2 = Sp.tile([128, 2 * TCH], f32, tag="S", name=f"S{tch}_{jp}")
                    for jj in range(2):
                        j = jp + jj
                        half = slice(64 * jj, 64 * (jj + 1))
                        ksl = kk2[j // 4][half, 128 * (j % 4) : 128 * (j % 4 + 1)]
                        lo = 128 * max(0, j - 4 * tch)  # causal: cols < lo unused
                        nc.tensor.matmul(
                            S2[:, TCH * jj + lo : TCH * (jj + 1)],
                            ksl,
                            qk2[tch][half, lo:TCH],
                            start=True,
                            stop=True,
                        )
                    ext = expp.tile(
                        [128, 2 * TCH], bf16, tag="ex", name=f"ex{tch}_{jp}"
                    )
                    if jp >= 4 * tch:
                        # diagonal pair: exp each half over its valid range only
                        for jj in range(2):
                            j = jp + jj
                            lo = TCH * jj + 128 * max(0, j - 4 * tch)
                            hi = TCH * (jj + 1)
                            nc.scalar.activation(
                                ext[:, lo:hi], S2[:, lo:hi], EXP, scale=SCALE
                            )
                    else:
                        nc.scalar.activation(ext[:], S2[:], EXP, scale=SCALE)
                    # causal: triangular mask multiply on diagonal blocks
                    for jj in range(2):
                        j = jp + jj
                        rel = j - 4 * tch
                        if rel >= 0:
                            a = TCH * jj + 128 * rel
                            nc.gpsimd.tensor_mul(
                                ext[:, a : a + 128], ext[:, a : a + 128], mask_t[:]
                            )
                    # PV accumulation (adds softmax-denominator row via ones col)
                    for jj in range(2):
                        j = jp + jj
                        lo = 128 * max(0, j - 4 * tch)
                        nc.tensor.matmul(
                            acc[:, lo:TCH] if j > 0 else acc[:, :],
                            v1[j][:],
                            ext[:, TCH * jj + lo : TCH * (jj + 1)],
                            start=(j == 0),
                            stop=(j == jmax),
                            skip_group_check=True,
                        )
                    # emit a slice of next-chunk projection work
                    for _ in range(per_pair):
                        if pending:
                            pending.pop(0)()
                for s in pending:
                    s()

                # ======== epilogue: normalize + transpose + DMA out ========
                oT = epip.tile([65, TCH], f32, tag="oT", name=f"oT{tch}")
                nc.vector.tensor_copy(oT[:], acc[:])
                for i in range(4):
                    Pe = miscp.tile([128, TCH], f32, tag="misc", name=f"Pe{tch}_{i}")
                    nc.tensor.transpose(
                        Pe[0:128, 0:65],
                        oT[:, 128 * i : 128 * (i + 1)],
                        idf_t[0:65, 0:65],
                    )
                    rec = epip.tile([128, 1], f32, tag="rec", name=f"rec{tch}_{i}")
                    nc.vector.reciprocal(rec[:], Pe[0:128, 64:65])
                    ot = epip.tile([128, H], f32, tag="ot", name=f"ot{tch}_{i}")
                    nc.vector.tensor_scalar_mul(ot[:], Pe[0:128, 0:64], rec[:])
                    r0 = TCH * tch + 128 * i
                    nc.sync.dma_start(out=out_d[r0 : r0 + 128, :], in_=ot[:])

    nc.compile()
    return nc


def _get_nc():
    if "nc" not in _CACHE:
        _CACHE["nc"] = _build()
    return _CACHE["nc"]


def _tile_w(w):
    """[C, F] -> [128, N_CT*F] with c-tile-major column blocks."""
    Cdim, F = w.shape
    return np.ascontiguousarray(
        w.reshape(Cdim // 128, 128, F).transpose(1, 0, 2).reshape(128, -1)
    )


def _host_inputs(x, w_q, w_k, w_v):
    bf = ml_dtypes.bfloat16
    x = np.asarray(x, dtype=np.float32)
    wqk = np.concatenate(
        [np.asarray(w_q, np.float32), np.asarray(w_k, np.float32)], 1
    )
    wv = np.asarray(w_v, np.float32)
    wqk_tiled = _tile_w(wqk).astype(bf)
    wv_tiled = _tile_w(wv).astype(bf)
    mask = np.triu(np.ones((128, 128), np.float32)).astype(bf)
    idf = np.eye(128, dtype=np.float32)
    idb = np.eye(128, dtype=np.float32).astype(bf)
    in_maps = []
    for i in range(N_CORES):
        in_maps.append(
            {
                "xT": np.ascontiguousarray(x[i].T).astype(bf),
                "wqk": wqk_tiled,
                "wv": wv_tiled,
                "maskb": mask,
                "idf": idf,
                "idb": idb,
            }
        )
    return in_maps


def run(x, w_q, w_k, w_v, trace=False, **trace_kwargs):
    from concourse.bass_utils import run_bass_kernel_spmd

    nc = _get_nc()
    in_maps = _host_inputs(x, w_q, w_k, w_v)
    res = run_bass_kernel_spmd(
        nc, in_maps, core_ids=list(range(N_CORES)), trace=trace, **trace_kwargs
    )
    out = np.stack([np.asarray(res.results[i]["out"]) for i in range(N_CORES)])
    return out.astype(np.float32), res


def kernel(x, w_q, w_k, w_v):
    out, _ = run(x, w_q, w_k, w_v, trace=False)
    return out


# revision 15
# speedup vs baseline: 1.4021x; 1.0121x over previous
"""Distributed Trainium2 kernel for a single attention head.

Problem: x:[8,2048,1024] f32, w_q/w_k/w_v:[1024,64] f32
  q,k,v = x@w ; scores = (q k^T)/sqrt(1024) causal-masked; out = softmax(scores)@v

Sharding: data-parallel over batch B=8 across the 8 NeuronCores (one batch
element per core, weights replicated, no collectives).

Per-core dataflow (T=2048, C=1024, H=64):
  - host ships x^T [C,T] in bf16 (layout marshalling), pre-tiled packed
    w_qk / w_v (bf16), a triangular mask tile, and identities for transposes.
  - projections with weights stationary (bf16): qT,kT duplicated on both
    partition halves [128,T] so scores can run 2x row-tiled; vT [64,T].
  - scores computed TRANSPOSED per s-tile: S[s,t] = kT_slice.T @ qT (K=64)
    so the PV contraction over s has s on partitions. Two s-tiles run
    concurrently in PE row-groups 0/1 (tile_position row packing).
  - exp on ScalarE with scale=1/32 folded in (no max-subtraction needed:
    |scores|<~2), output cast to bf16.
  - causal: only j<=t blocks computed; triangular mask multiply on diagonal
    blocks runs on GpSimd (otherwise idle).
  - PV: out^T[h,t] accumulated over s-tiles with lhsT = [v | 1] so row 64 of
    the accumulator is the softmax denominator (fused row-sum).
  - epilogue: TensorE transpose back to [t,h], multiply by reciprocal
    denominator on VectorE, DMA out (f32).
  - next-chunk projections are emission-interleaved between attention pairs
    so the PE stream stays dense (HAM stays warm) and input DMAs are spread
    over the 3 DMA-capable queues (sync/gpsimd/scalar).
"""

import os
import sys

import numpy as np

for p in ("/opt/trn_rl_repo",):
    if p not in sys.path and os.path.isdir(p):
        sys.path.insert(0, p)

import ml_dtypes  # noqa: E402

B, T, C, H = 8, 2048, 1024, 64
N_CORES = 8
TCH = 512                  # t-chunk (columns per PSUM bank of f32)
N_CHUNK = T // TCH         # 4
N_CT = C // 128            # 8 contraction tiles
SCALE = float(C) ** -0.5   # 1/32

_CACHE = {}


def _build():
    """Build + compile the SPMD Bass graph (same graph on all 8 cores)."""
    import concourse.bass as bass
    import concourse.mybir as mybir
    import concourse.tile as tile
    from concourse import bacc

    f32 = mybir.dt.float32
    bf16 = mybir.dt.bfloat16
    EXP = mybir.ActivationFunctionType.Exp

    nc = bacc.Bacc(
        "TRN2", target_bir_lowering=False, debug=False, num_devices=N_CORES
    )

    # host ships weights pre-tiled: [128, N_CT*free] with c-tile-major columns
    xT_d = nc.dram_tensor("xT", [C, T], bf16, kind="ExternalInput")
    wqk_d = nc.dram_tensor("wqk", [128, N_CT * 128], bf16, kind="ExternalInput")
    wv_d = nc.dram_tensor("wv", [128, N_CT * H], bf16, kind="ExternalInput")
    mask_d = nc.dram_tensor("maskb", [128, 128], bf16, kind="ExternalInput")  # -1e5 strict-lower-tri
    idf_d = nc.dram_tensor("idf", [128, 128], f32, kind="ExternalInput")
    idb_d = nc.dram_tensor("idb", [128, 128], bf16, kind="ExternalInput")
    out_d = nc.dram_tensor("out", [T, H], f32, kind="ExternalOutput")

    with tile.TileContext(nc) as tc:
        with (
            tc.tile_pool(name="const", bufs=1) as constp,
            tc.tile_pool(name="xTp", bufs=1) as xTp,
            tc.tile_pool(name="qkp", bufs=1) as qkp,
            tc.tile_pool(name="v1p", bufs=1) as v1p,
            tc.tile_pool(name="exp", bufs=6) as expp,
            tc.tile_pool(name="epi", bufs=3) as epip,
            tc.tile_pool(name="Sp", bufs=2, space="PSUM") as Sp,
            tc.tile_pool(name="accp", bufs=1, space="PSUM") as accp,
            tc.tile_pool(name="miscp", bufs=3, space="PSUM") as miscp,
        ):
            # ---- weights first (contiguous per-partition layout) ----
            wqk_t = constp.tile([128, N_CT, 128], bf16, tag="wqk", name="wqk_t")
            nc.sync.dma_start(
                out=wqk_t[:], in_=wqk_d[:].rearrange("p (n m) -> p n m", n=N_CT)
            )
            wv_t = constp.tile([128, N_CT, H], bf16, tag="wv", name="wv_t")
            nc.scalar.dma_start(
                out=wv_t[:], in_=wv_d[:].rearrange("p (n m) -> p n m", n=N_CT)
            )

            # ---- x^T tiles: chunk 0 first, 3 parallel DMA queues ----
            dma_engines = [nc.sync, nc.gpsimd, nc.scalar]
            xt = {}

            def emit_x_dma(t, c):
                xx = xTp.tile([128, TCH], bf16, tag=f"x{c}_{t}", name=f"x{c}_{t}")
                eng = dma_engines[(t * N_CT + c) % 3]
                eng.dma_start(
                    out=xx[:],
                    in_=xT_d[128 * c : 128 * (c + 1), TCH * t : TCH * (t + 1)],
                )
                xt[c, t] = xx

            for c in range(N_CT):
                emit_x_dma(0, c)

            # small constants after chunk-0 x
            mask_t = constp.tile([128, 128], bf16, tag="mask", name="mask_t")
            nc.gpsimd.dma_start(out=mask_t[:], in_=mask_d[:])
            idf_t = constp.tile([128, 128], f32, tag="idf", name="idf_t")
            nc.gpsimd.dma_start(out=idf_t[:], in_=idf_d[:])
            idb_t = constp.tile([128, 128], bf16, tag="idb", name="idb_t")
            nc.gpsimd.dma_start(out=idb_t[:], in_=idb_d[:])

            for t in range(1, N_CHUNK):
                for c in range(N_CT):
                    emit_x_dma(t, c)

            qk2 = {}   # [128, TCH]: qT duplicated on both partition halves
            kk2 = {}   # [128, TCH]: kT duplicated on both partition halves
            v1 = {}

            def proj_steps(tch):
                """Emission thunks for chunk `tch` projections + v1 build."""
                steps = []
                state = {}

                def qk_mm(c):
                    def f():
                        if c == 0:
                            state["S"] = miscp.tile(
                                [128, TCH], f32, tag="misc", name=f"Sqk{tch}"
                            )
                        nc.tensor.matmul(
                            state["S"][:, :],
                            wqk_t[:, c, :],
                            xt[c, tch][:],
                            start=(c == 0),
                            stop=(c == N_CT - 1),
                            skip_group_check=True,
                        )
                    return f

                def qk_out():
                    S = state["S"]
                    q2 = qkp.tile([128, TCH], bf16, tag=f"q2_{tch}", name=f"q2_{tch}")
                    k2 = qkp.tile([128, TCH], bf16, tag=f"k2_{tch}", name=f"k2_{tch}")
                    nc.vector.tensor_copy(q2[0:64, :], S[0:64, :])
                    nc.vector.tensor_copy(k2[0:64, :], S[64:128, :])
                    nc.gpsimd.dma_start(out=q2[64:128, :], in_=q2[0:64, :])
                    nc.gpsimd.dma_start(out=k2[64:128, :], in_=k2[0:64, :])
                    qk2[tch] = q2
                    kk2[tch] = k2

                def v_mm(c):
                    def f():
                        if c == 0:
                            state["Pv"] = miscp.tile(
                                [128, TCH], f32, tag="misc", name=f"Pv{tch}"
                            )
                        nc.tensor.matmul(
                            state["Pv"][0:64, :],
                            wv_t[:, c, :],
                            xt[c, tch][:],
                            start=(c == 0),
                            stop=(c == N_CT - 1),
                            skip_group_check=True,
                        )
                    return f

                def v_out():
                    vTt = qkp.tile([64, TCH], bf16, tag=f"vT{tch}", name=f"vT{tch}")
                    nc.vector.tensor_copy(vTt[:], state["Pv"][0:64, :])
                    state["vT"] = vTt

                def v1_build(i):
                    def f():
                        j = 4 * tch + i
                        Pt = miscp.tile([128, TCH], bf16, tag="misc", name=f"Pt{j}")
                        nc.tensor.transpose(
                            Pt[0:128, 0:64],
                            state["vT"][:, 128 * i : 128 * (i + 1)],
                            idb_t[0:64, 0:64],
                        )
                        v1t = v1p.tile(
                            [128, 65], bf16, tag=f"v1_{j}", name=f"v1_{j}"
                        )
                        nc.vector.tensor_copy(v1t[:, 0:64], Pt[0:128, 0:64])
                        nc.vector.memset(v1t[:, 64:65], 1.0)
                        v1[j] = v1t
                    return f

                for c in range(N_CT):
                    steps.append(qk_mm(c))
                steps.append(qk_out)
                for c in range(N_CT):
                    steps.append(v_mm(c))
                steps.append(v_out)
                for i in range(4):
                    steps.append(v1_build(i))
                return steps

            # chunk 0 projections up front
            for s in proj_steps(0):
                s()

            for tch in range(N_CHUNK):
                # interleave next chunk's projection emission between pairs
                pending = proj_steps(tch + 1) if tch + 1 < N_CHUNK else []
                jmax = 4 * tch + 3
                pairs = list(range(0, jmax + 1, 2))
                per_pair = -(-len(pending) // len(pairs)) if pending else 0

                acc = accp.tile([65, TCH], f32, tag="acc", name=f"acc{tch}")
                for pi, jp in enumerate(pairs):
                    S2 = Sp.tile([128, 2 * TCH], f32, tag="S", name=f"S{tch}_{jp}")
                    for jj in range(2):
                        j = jp + jj
                        half = slice(64 * jj, 64 * (jj + 1))
                        ksl = kk2[j // 4][half, 128 * (j % 4) : 128 * (j % 4 + 1)]
                        rel = j - 4 * tch
                        # diagonal s-tiles: full-range scores, then add the
                        # -1e5 strict-lower-tri mask into the diagonal block
                        # via an identity matmul (keeps PE the only producer)
                        nc.tensor.matmul(
                            S2[:, TCH * jj : TCH * (jj + 1)],
                            ksl,
                            qk2[tch][half, :],
                            start=True,
                            stop=(rel < 0),
                            skip_group_check=True,
                        )
                        if rel >= 0:
                            a = TCH * jj + 128 * rel
                            nc.tensor.matmul(
                                S2[:, a : a + 128],
                                idb_t[:],
                                mask_t[:],
                                start=False,
                                stop=True,
                                skip_group_check=True,
                            )
                    ext = expp.tile(
                        [128, 2 * TCH], bf16, tag="ex", name=f"ex{tch}_{jp}"
                    )
                    nc.scalar.activation(ext[:], S2[:], EXP, scale=SCALE)
                    # PV accumulation (adds softmax-denominator row via ones col)
                    for jj in range(2):
                        j = jp + jj
                        lo = 128 * max(0, j - 4 * tch)
                        nc.tensor.matmul(
                            acc[:, lo:TCH] if j > 0 else acc[:, :],
                            v1[j][:],
                            ext[:, TCH * jj + lo : TCH * (jj + 1)],
                            start=(j == 0),
                            stop=(j == jmax),
                            skip_group_check=True,
                        )
                    # emit a slice of next-chunk projection work
                    for _ in range(per_pair):
                        if pending:
                            pending.pop(0)()
                for s in pending:
                    s()

                # ======== epilogue: normalize + transpose + DMA out ========
                oT = epip.tile([65, TCH], f32, tag="oT", name=f"oT{tch}")
                nc.vector.tensor_copy(oT[:], acc[:])
                for i in range(4):
                    Pe = miscp.tile([128, TCH], f32, tag="misc", name=f"Pe{tch}_{i}")
                    nc.tensor.transpose(
                        Pe[0:128, 0:65],
                        oT[:, 128 * i : 128 * (i + 1)],
                        idf_t[0:65, 0:65],
                    )
                    rec = epip.tile([128, 1], f32, tag="rec", name=f"rec{tch}_{i}")
                    nc.vector.reciprocal(rec[:], Pe[0:128, 64:65])
                    ot = epip.tile([128, H], f32, tag="ot", name=f"ot{tch}_{i}")
                    nc.vector.tensor_scalar_mul(ot[:], Pe[0:128, 0:64], rec[:])
                    r0 = TCH * tch + 128 * i
                    nc.sync.dma_start(out=out_d[r0 : r0 + 128, :], in_=ot[:])

    nc.compile()
    return nc


def _get_nc():
    if "nc" not in _CACHE:
        _CACHE["nc"] = _build()
    return _CACHE["nc"]


def _tile_w(w):
    """[C, F] -> [128, N_CT*F] with c-tile-major column blocks."""
    Cdim, F = w.shape
    return np.ascontiguousarray(
        w.reshape(Cdim // 128, 128, F).transpose(1, 0, 2).reshape(128, -1)
    )


def _host_inputs(x, w_q, w_k, w_v):
    bf = ml_dtypes.bfloat16
    x = np.asarray(x, dtype=np.float32)
    wqk = np.concatenate(
        [np.asarray(w_q, np.float32), np.asarray(w_k, np.float32)], 1
    )
    wv = np.asarray(w_v, np.float32)
    wqk_tiled = _tile_w(wqk).astype(bf)
    wv_tiled = _tile_w(wv).astype(bf)
    # additive causal mask for transposed-score diag blocks: kill s > t
    mask = (np.tril(np.ones((128, 128), np.float32), -1) * -1e5).astype(bf)
    idf = np.eye(128, dtype=np.float32)
    idb = np.eye(128, dtype=np.float32).astype(bf)
    in_maps = []
    for i in range(N_CORES):
        in_maps.append(
            {
                "xT": np.ascontiguousarray(x[i].T).astype(bf),
                "wqk": wqk_tiled,
                "wv": wv_tiled,
                "maskb": mask,
                "idf": idf,
                "idb": idb,
            }
        )
    return in_maps


def run(x, w_q, w_k, w_v, trace=False, **trace_kwargs):
    from concourse.bass_utils import run_bass_kernel_spmd

    nc = _get_nc()
    in_maps = _host_inputs(x, w_q, w_k, w_v)
    res = run_bass_kernel_spmd(
        nc, in_maps, core_ids=list(range(N_CORES)), trace=trace, **trace_kwargs
    )
    out = np.stack([np.asarray(res.results[i]["out"]) for i in range(N_CORES)])
    return out.astype(np.float32), res


def kernel(x, w_q, w_k, w_v):
    out, _ = run(x, w_q, w_k, w_v, trace=False)
    return out
